# revision 1
# baseline (speedup 1.0000x reference)
"""DeepFM forward kernel for Trainium2, data-parallel over 8 NeuronCores.

Math refactor vs the straightforward DeepFM graph:
  1. Tower dense outputs are never materialized: W1 is folded into the
     tower weights host-side (z1 = xm @ (Wm_d@W1[:256]) + xu @ (Wu_d@W1[256:])),
     and the FM interaction sum collapses to 16 fold sums per tower.
  2. The FM sum uses the polarization identity sum fold_m.fold_u + add =
     sum (p^2 - q^2)/4 + a (p/q linear in x; the additive term rides two
     rows as ((a+1)/2)^2 - ((a-1)/2)^2 = a): one matmul accumulation chain
     plus one scalar-engine Square, folded into the final matmul.

Precision scheme (fp8 DoubleRow everywhere the PE is hot):
  - Inputs ship as x8 = fp8(x) plus the scaled residual r8 = fp8(16*(x-x8)).
    Combined they carry ~12 mantissa bits at the same 4MB as one bf16 copy.
  - z1 matmuls: fp8 DoubleRow (K=256 per matmul, 2 multiplies/cell/cycle),
    weights scaled x16 into e4m3 range; the relu's scale=1/16 undoes it.
  - The precision-critical FM/extras chain runs three DoubleRow chains:
    F8^T x8 + (F8/16)^T r8 + Fr8^T x8  (weight-quantization residual Fr8),
    recovering bf16-level accuracy; the Square's scale=1/G undoes the
    weight scale G. MLP2/final stay bf16 on on-chip operands.
All matmuls run in uniform 128x128 PE tiling mode (narrow lhsT zero-padded
to M=128) so the array never pays a mode-switch drain.
"""

import numpy as np
import ml_dtypes

import concourse.bacc as bacc
import concourse.bass as bass  # noqa: F401
import concourse.mybir as mybir
import concourse.tile as tile
from concourse.bass_utils import run_bass_kernel_spmd

N_CORES = 8
B_FULL = 16384
R = B_FULL // N_CORES  # 2048 rows per core
F = 512                # input features per tower
KC = F // 128          # 4 contraction chunks per tower
NT = 512               # batch tile on the free dim
NTILES = R // NT       # 4
NX = 34                # extras rows: p(16) + q(16) + a-rows(2)
N_WARM = 23            # PE pre-warm matmuls (N=256): enough sustained PE
                       # busy to cross the ~3.4us HAM activity window BEFORE
                       # the chain ends (with margin for window phase) — once
                       # the clock gate opens, short idle gaps are harmless
XW = 128               # extras lhsT zero-padded to M=128 (uniform PE mode)
XT_COLS = 2 * KC * NT  # per-tile input cols (both towers)

F32 = mybir.dt.float32
BF16 = mybir.dt.bfloat16
E4M3 = mybir.dt.float8e4

Z1_COLS = 16 * 128     # fp8 z1 blob: [xm-g0 | xm-g1 | xu-g0 | xu-g1]
Z1_SCALE = 16.0
XG = 4.0               # extras weight scale (undone by the Square's scale)
RS = 16.0              # x-residual scale: r8 = fp8(RS * (x - x8))
NCH = 3                # extras chains: F8^T x8, (F8/RS)^T r8, Fr8^T x8

# bf16 weight-pack column offsets (W2 | W3 pad | WQ pad)
W2_OFF = 0
W3_OFF = 2 * 128
WQ_OFF = W3_OFF + 128
WCOLS = WQ_OFF + 128

# fp32 bias-pack column indices ([128, BCOLS])
B1A, B1B, BX, B2C = range(4)
BCOLS = 4


def _chunk3(Wext, kc=8):
    """[K, M] -> [128, kc, M]: chunk k = rows k*128..(k+1)*128."""
    m = Wext.shape[1]
    return np.ascontiguousarray(Wext.reshape(kc, 128, m).transpose(1, 0, 2))


def _col(vec):
    out = np.zeros((128, 1), np.float32)
    out[: len(vec), 0] = vec
    return out


def _pack_weights(Wm, bm, Wu, bu, W1, b1, W2, b2, W3, b3):
    f64 = np.float64
    fp8 = lambda a: np.asarray(a, ml_dtypes.float8_e4m3).astype(f64)
    Wm, bm, Wu, bu = Wm.astype(f64), bm.astype(f64), Wu.astype(f64), bu.astype(f64)
    W1, b1, W2, b2 = W1.astype(f64), b1.astype(f64), W2.astype(f64), b2.astype(f64)
    b3v = float(np.asarray(b3, f64).reshape(-1)[0])

    Am = Wm[:, :256] @ W1[:256, :]
    Au = Wu[:, :256] @ W1[256:, :]
    b1p = b1 + bm[:256] @ W1[:256, :] + bu[:256] @ W1[256:, :]

    FWm = Wm[:, :256].reshape(F, 16, 16).sum(axis=1)
    FWu = Wu[:, :256].reshape(F, 16, 16).sum(axis=1)
    fbm = bm[:256].reshape(16, 16).sum(axis=0)
    fbu = bu[:256].reshape(16, 16).sum(axis=0)
    awm, awu = Wm[:, 256], Wu[:, 256]
    A = bm[256] + bu[256] + b3v
    Xm = np.concatenate([FWm, FWm, awm[:, None] / 2, awm[:, None] / 2], axis=1)
    Xu = np.concatenate([FWu, -FWu, awu[:, None] / 2, awu[:, None] / 2], axis=1)
    xbias = np.concatenate([fbm + fbu, fbm - fbu, [(A + 1) / 2], [(A - 1) / 2]])
    wq = np.concatenate([np.full(16, 0.25), np.full(16, -0.25), [1.0, -1.0]])

    # fp8 z1 blob [128, 2(half), 2(g), KC, 128], scaled x16
    amc, auc = _chunk3(Am, KC), _chunk3(Au, KC)  # [128, 4, 256]
    w8 = np.stack(
        [
            np.stack([amc[:, :, :128], amc[:, :, 128:]], axis=1),
            np.stack([auc[:, :, :128], auc[:, :, 128:]], axis=1),
        ],
        axis=1,
    )  # [128, half, g, KC, 128]
    w8 = (w8 * Z1_SCALE).astype(ml_dtypes.float8_e4m3)

    # fp8 extras blob [128, NCH, 8, XW]: chains c1=F8, c2=F8/RS, c3=Fr8
    XWmat = np.concatenate([Xm, Xu], axis=0)          # [1024, 34]
    XWpad = np.zeros((2 * F, XW), f64)
    XWpad[:, :NX] = XWmat
    F8 = fp8(XWpad * XG)
    Fr = XWpad * XG - F8
    wx8 = np.stack(
        [_chunk3(F8), _chunk3(F8 / RS), _chunk3(Fr)], axis=1
    )  # [128, 3, 8, XW]
    wx8 = wx8.astype(ml_dtypes.float8_e4m3)

    w3_pad = np.zeros((128, 128), f64)
    w3_pad[:, 0] = np.asarray(W3, f64).reshape(128) / Z1_SCALE
    wq_pad = np.zeros((128, 128), f64)
    wq_pad[:NX, 0] = wq
    w28 = (_chunk3(W2, 2) * Z1_SCALE).astype(ml_dtypes.float8_e4m3)
    wp = np.concatenate(
        [_chunk3(W2, 2).reshape(128, 256), w3_pad, wq_pad], axis=1
    )
    assert wp.shape == (128, WCOLS), wp.shape
    bp = np.concatenate(
        [_col(b1p[:128]), _col(b1p[128:]), _col(xbias), _col(b2 * Z1_SCALE)], axis=1
    )
    return (
        np.ascontiguousarray(w8.reshape(128, Z1_COLS)),
        np.ascontiguousarray(w28.reshape(128, 256)),
        np.ascontiguousarray(wx8.reshape(128, NCH * 8 * XW)),
        np.ascontiguousarray(wp.astype(ml_dtypes.bfloat16)),
        np.ascontiguousarray(bp.astype(np.float32)),
    )


def _build_bass():
    nc = bacc.Bacc()
    x8 = nc.dram_tensor("x8", [128, NTILES * XT_COLS], E4M3, kind="ExternalInput")
    r8 = nc.dram_tensor("r8", [128, NTILES * XT_COLS], E4M3, kind="ExternalInput")
    w8d = nc.dram_tensor("w8", [128, Z1_COLS], E4M3, kind="ExternalInput")
    wx8d = nc.dram_tensor("wx8", [128, NCH * 8 * XW], E4M3, kind="ExternalInput")
    w28d = nc.dram_tensor("w28", [128, 256], E4M3, kind="ExternalInput")
    wpd = nc.dram_tensor("wp", [128, WCOLS], BF16, kind="ExternalInput")
    bpd = nc.dram_tensor("bp", [128, BCOLS], F32, kind="ExternalInput")
    out = nc.dram_tensor("out", [1, R], F32, kind="ExternalOutput")

    relu = mybir.ActivationFunctionType.Relu
    square = mybir.ActivationFunctionType.Square
    DR = mybir.MatmulPerfMode.DoubleRow

    with tile.TileContext(nc) as tc:
        with (
            tc.tile_pool(name="wpool", bufs=1) as wpool,
            tc.tile_pool(name="xpool", bufs=1) as xpool,
            tc.tile_pool(name="dpool", bufs=1) as dpool,
            tc.tile_pool(name="opool", bufs=1) as opool,
            tc.tile_pool(name="psz", bufs=3, space="PSUM") as psz,
            tc.tile_pool(name="psx", bufs=2, space="PSUM") as psx,
            tc.tile_pool(name="psm", bufs=1, space="PSUM") as psm,
            tc.tile_pool(name="psf", bufs=2, space="PSUM") as psf,
        ):
            # PE pre-warm (see N_WARM note)
            wgar = wpool.tile([128, NT], BF16)
            nc.gpsimd.memset(wgar, 0.0)
            for _ in range(N_WARM):
                pw = psz.tile([128, NT], F32, name="ps_z1")
                nc.tensor.matmul(
                    pw[:, :256], wgar[:, :128], wgar[:, :256],
                    start=True, stop=True,
                )

            # weights on the scalar ring in consumption order
            H = Z1_COLS // 2
            w8m = wpool.tile([128, 2, KC, 128], E4M3)
            nc.scalar.dma_start(out=w8m, in_=w8d[:, :H])
            w8u = wpool.tile([128, 2, KC, 128], E4M3)
            nc.scalar.dma_start(out=w8u, in_=w8d[:, H:])
            b = wpool.tile([128, BCOLS], F32)
            nc.scalar.dma_start(out=b, in_=bpd[:, :])
            wrm = wpool.tile([128, WCOLS], BF16)
            nc.scalar.dma_start(out=wrm, in_=wpd[:, :])
            wx8 = wpool.tile([128, NCH, 8, XW], E4M3)
            nc.scalar.dma_start(out=wx8, in_=wx8d[:, :])
            w28 = wpool.tile([128, 2, 128], E4M3)
            nc.scalar.dma_start(out=w28, in_=w28d[:, :])
            out_sb = opool.tile([1, NTILES * NT], F32)

            x8r = x8.rearrange("p (t w c n) -> p t w c n", t=NTILES, w=2, c=KC, n=NT)
            r8r = r8.rearrange("p (t w c n) -> p t w c n", t=NTILES, w=2, c=KC, n=NT)

            # inputs on the sync ring: x8 then r8 per tile (consumption
            # order); tile-0 x8 split per tower for the earliest start
            x80m = xpool.tile([128, KC, NT], E4M3)
            nc.sync.dma_start(out=x80m, in_=x8r[:, 0, 0])
            x80u = xpool.tile([128, KC, NT], E4M3)
            nc.sync.dma_start(out=x80u, in_=x8r[:, 0, 1])
            r80 = xpool.tile([128, 2, KC, NT], E4M3)
            nc.sync.dma_start(out=r80, in_=r8r[:, 0])
            x8ts = [(x80m, x80u)]
            r8ts = [(r80[:, 0], r80[:, 1])]
            for t in range(1, NTILES):
                x8t = xpool.tile([128, 2, KC, NT], E4M3, name=f"x8_{t}")
                nc.sync.dma_start(out=x8t, in_=x8r[:, t])
                x8ts.append((x8t[:, 0], x8t[:, 1]))
                r8t = xpool.tile([128, 2, KC, NT], E4M3, name=f"r8_{t}")
                nc.sync.dma_start(out=r8t, in_=r8r[:, t])
                r8ts.append((r8t[:, 0], r8t[:, 1]))

            h1s, sqs, h2s = {}, {}, {}

            def emit_z1(t):
                # fp8 DoubleRow: each matmul contracts K=256 (two chunks as
                # a [128, 2, *] AP). xm feeds both output groups first so
                # the xu input can land meanwhile.
                xm8, xu8 = x8ts[t]
                pss = []
                for g in range(2):
                    pss.append(psz.tile([128, NT], F32, name="ps_z1"))
                for half, xf in ((0, xm8), (1, xu8)):
                    if half == 1 and t == 0:
                        # filler matmuls bridge the tile-0 xu-input wait
                        # (always >=2.2us): they keep the HAM clock-gate
                        # warm when the wait stretches past the idle-window
                        # threshold, preventing a 2us cold-clock cascade
                        for _ in range(5):
                            pw = psm.tile([128, NT], F32, name="ps_m")
                            nc.tensor.matmul(
                                pw[:, :256], wgar[:, :128], wgar[:, :256],
                                start=True, stop=True,
                            )
                    w8t = (w8m, w8u)[half]
                    for g in range(2):
                        for p in range(KC // 2):
                            nc.tensor.matmul(
                                pss[g],
                                w8t[:, g, 2 * p : 2 * p + 2, :],
                                xf[:, 2 * p : 2 * p + 2, :],
                                start=(half == 0 and p == 0),
                                stop=(half == 1 and p == KC // 2 - 1),
                                perf_mode=DR,
                            )
                        if half == 1:
                            if t not in h1s:
                                h1s[t] = dpool.tile(
                                    [128, 2, NT], E4M3, name=f"h1_{t}"
                                )
                            nc.scalar.activation(
                                out=h1s[t][:, g, :], in_=pss[g], func=relu,
                                bias=b[:, g : g + 1], scale=1.0 / Z1_SCALE,
                            )

            def emit_extras(t):
                # three DoubleRow chains accumulate G * (XW^T x) in fp32:
                # F8^T x8 + (F8/RS)^T r8 + Fr8^T x8; the Square's
                # scale=1/G recovers the true pre-activation.
                ps = psx.tile([128, NT], F32, name="ps_x")
                first, last = (0, 0, 0), (NCH - 1, 1, KC // 2 - 1)
                for c in range(NCH):
                    src = r8ts[t] if c == 1 else x8ts[t]
                    for tw in range(2):
                        xf = src[tw]
                        for p in range(KC // 2):
                            nc.tensor.matmul(
                                ps,
                                wx8[:, c, tw * KC + 2 * p : tw * KC + 2 * p + 2, :],
                                xf[:, 2 * p : 2 * p + 2, :],
                                start=((c, tw, p) == first),
                                stop=((c, tw, p) == last),
                                perf_mode=DR,
                            )
                sq = dpool.tile([128, NT], BF16, name=f"sq_{t}")
                nc.scalar.activation(
                    out=sq, in_=ps, func=square,
                    bias=b[:, BX : BX + 1], scale=1.0 / XG,
                )
                sqs[t] = sq

            def emit_mlp2(t):
                # one fp8 DoubleRow matmul (K=256); W2 scaled x16, undone by
                # the relu's scale
                ps = psm.tile([128, NT], F32, name="ps_m")
                nc.tensor.matmul(
                    ps, w28[:, 0:2, :], h1s[t][:, 0:2, :],
                    start=True, stop=True, perf_mode=DR,
                )
                h2 = dpool.tile([128, NT], BF16, name=f"h2_{t}")
                nc.vector.tensor_scalar(
                    out=h2, in0=ps, scalar1=b[:, B2C : B2C + 1], scalar2=0.0,
                    op0=mybir.AluOpType.add, op1=mybir.AluOpType.max,
                )
                h2s[t] = h2

            def emit_final(t):
                # sq matmul first: its operand is ready well before h2
                ps = psf.tile([128, NT], F32, name="ps_f")
                nc.tensor.matmul(
                    ps, wrm[:, WQ_OFF : WQ_OFF + 128], sqs[t],
                    start=True, stop=False,
                )
                nc.tensor.matmul(
                    ps, wrm[:, W3_OFF : W3_OFF + 128], h2s[t],
                    start=False, stop=True,
                )
                n0 = t * NT
                # per-tile staging columns: copy_t never WAR-serializes
                # against the previous tile's still-reading out DMA
                ob = out_sb[0:1, n0 : n0 + NT]
                nc.scalar.copy(ob, ps[0:1])
                nc.sync.dma_start(out=out[:, n0 : n0 + NT], in_=ob)

            for t in range(NTILES):
                emit_z1(t)
                if t > 0:
                    emit_mlp2(t - 1)
                emit_extras(t)
                if t > 0:
                    emit_final(t - 1)
            emit_mlp2(NTILES - 1)
            emit_final(NTILES - 1)
    nc.finalize()
    return nc


def _pack_x(xmT_core, xuT_core):
    """2x [512, 2048] fp32 -> ([128, .] fp8 x8, [128, .] fp8 r8)."""
    ym = xmT_core.reshape(KC, 128, NTILES, NT).transpose(1, 2, 0, 3)
    yu = xuT_core.reshape(KC, 128, NTILES, NT).transpose(1, 2, 0, 3)
    y = np.stack([ym, yu], axis=2).reshape(128, NTILES * XT_COLS)
    x8 = y.astype(ml_dtypes.float8_e4m3)
    r8 = ((y - x8.astype(np.float32)) * RS).astype(ml_dtypes.float8_e4m3)
    return np.ascontiguousarray(x8), np.ascontiguousarray(r8)


_NC_CACHE = []


def kernel(movie_vectors, user_vectors, Wm, bm, Wu, bu, W1, b1, W2, b2, W3, b3):
    movie_vectors = np.asarray(movie_vectors, np.float32)
    user_vectors = np.asarray(user_vectors, np.float32)
    w8, w28, wx8, wp, bp = _pack_weights(
        np.asarray(Wm, np.float32), np.asarray(bm, np.float32),
        np.asarray(Wu, np.float32), np.asarray(bu, np.float32),
        np.asarray(W1, np.float32), np.asarray(b1, np.float32),
        np.asarray(W2, np.float32), np.asarray(b2, np.float32),
        np.asarray(W3, np.float32), np.asarray(b3, np.float32),
    )
    xmT = movie_vectors.T  # [512, 16384]
    xuT = user_vectors.T

    if not _NC_CACHE:
        _NC_CACHE.append(_build_bass())
    nc = _NC_CACHE[0]

    in_maps = []
    for c in range(N_CORES):
        sl = slice(c * R, (c + 1) * R)
        x8a, r8a = _pack_x(xmT[:, sl], xuT[:, sl])
        in_maps.append(
            {
                "x8": x8a, "r8": r8a, "w8": w8, "w28": w28,
                "wx8": wx8, "wp": wp, "bp": bp,
            }
        )
    res = run_bass_kernel_spmd(nc, in_maps, core_ids=list(range(N_CORES)))
    kernel.last_result = res
    return np.concatenate([r["out"].reshape(R, 1) for r in res.results], axis=0)



# revision 3
# speedup vs baseline: 1.4538x; 1.4538x over previous
"""DeepFM forward kernel for Trainium2, data-parallel over 8 NeuronCores.

Math refactor vs the straightforward DeepFM graph:
  1. Tower dense outputs are never materialized: W1 is folded into the
     tower weights host-side (z1 = xm @ (Wm_d@W1[:256]) + xu @ (Wu_d@W1[256:])),
     and the FM interaction sum collapses to 16 fold sums per tower.
  2. The FM sum uses the polarization identity sum fold_m.fold_u + add =
     sum (p^2 - q^2)/4 + a (p/q linear in x; the additive term rides two
     rows as ((a+1)/2)^2 - ((a-1)/2)^2 = a): one matmul accumulation chain
     plus one scalar-engine Square, folded into the final matmul.

Precision scheme (fp8 DoubleRow everywhere the PE is hot):
  - Inputs ship as x8 = fp8(x) plus the scaled residual r8 = fp8(16*(x-x8)).
    Combined they carry ~12 mantissa bits at the same 4MB as one bf16 copy.
  - z1 matmuls: fp8 DoubleRow (K=256 per matmul, 2 multiplies/cell/cycle),
    weights scaled x16 into e4m3 range; the relu's scale=1/16 undoes it.
  - The precision-critical FM/extras chain runs three DoubleRow chains:
    F8^T x8 + (F8/16)^T r8 + Fr8^T x8  (weight-quantization residual Fr8),
    recovering bf16-level accuracy; the Square's scale=1/G undoes the
    weight scale G. MLP2/final stay bf16 on on-chip operands.

Schedule notes (v3):
  - x8 loads ride the sync ring, r8 loads the gpsimd ring, weights the
    scalar ring: three independent DMA queues so descriptor issue never
    gates the stream (the single-ring version stalled the input stream
    ~2us waiting on ring credits).
  - One dma_start per input tile (4KB contiguous per partition line).
  - ALL weights ship as ONE byte-packed blob (3476B contiguous per
    partition line, bitcast views for the bf16/f32 regions): one fast
    dma_start instead of six small-line ones.  v2 shipped six separate
    small-line weight DMAs and the weight queue crawled at ~28GB/s,
    landing the extras weights at ~24us and starving the PE into a
    half-clock HAM cascade.
  - Extras lhsT packs M=48 (34 real cols + pad to keep the DoubleRow
    pair-dim step %16==0); the final W3/wq matmuls are M=1.  Weight
    stream is 435KB vs 822KB for the padded-M layout.
  - Single output DMA at the end (outputs are staged in SBUF).
"""

import numpy as np
import ml_dtypes

import concourse.bacc as bacc
import concourse.bass as bass  # noqa: F401
import concourse.mybir as mybir
import concourse.tile as tile
from concourse.bass_utils import run_bass_kernel_spmd

N_CORES = 8
B_FULL = 16384
R = B_FULL // N_CORES  # 2048 rows per core
F = 512                # input features per tower
KC = F // 128          # 4 contraction chunks per tower
NT = 512               # batch tile on the free dim
NTILES = R // NT       # 4
NX = 34                # extras rows: p(16) + q(16) + a-rows(2)
N_WARM = 14            # PE pre-warm matmuls (N=256): keep the PE busy from
                       # kernel start until tile-0 x8 lands (~5us in) so the
                       # HAM activity window accumulates without a break
XW = 48                # extras lhsT cols: 34 real + pad so the DoubleRow
                       # pair-dim step (=XW) stays %16==0
XT_COLS = 2 * KC * NT  # per-tile input cols (both towers)

F32 = mybir.dt.float32
BF16 = mybir.dt.bfloat16
E4M3 = mybir.dt.float8e4

Z1_COLS = 16 * 128     # fp8 z1 blob: [xm-g0 | xm-g1 | xu-g0 | xu-g1]
Z1_SCALE = 16.0
XG = 4.0               # extras weight scale (undone by the Square's scale)
RS = 16.0              # x-residual scale: r8 = fp8(RS * (x - x8))
NCH = 3                # extras chains: F8^T x8, (F8/RS)^T r8, Fr8^T x8

# bf16 weight-pack columns: [W3/Z1_SCALE | wq]
W3_COL = 0
WQ_COL = 1
WCOLS = 2

# fp32 bias-pack column indices ([128, BCOLS])
B1A, B1B, BX, B2C = range(4)
BCOLS = 4

# byte offsets within the secondary weight blob (per partition)
WX_BYTES = NCH * 4 * 2 * XW        # 1152
O_WX = 0
O_W28 = O_WX + WX_BYTES            # 1152
O_WP = O_W28 + 256                 # 1408 (2B-aligned for bf16)
O_BP = O_WP + 2 * WCOLS            # 1412 (4B-aligned for f32)
WBYTES = O_BP + 4 * BCOLS          # 1428

# the main stream: one DRAM tensor in strict consumption order, split
# fine at the head so z1(0)'s first matmuls start after only 384KB:
# [w8m | x0m | w8u | x0u | r0 | x1 | r1 | x2 | r2 | x3 | r3]
TB = 2 * KC * NT                   # 4096B: one tile's x (or r), per partition
HB = TB // 2                       # 2048B: one tower half
W8H = Z1_COLS // 2                 # 1024B: one tower's z1 weights
S_W8M = 0
S_T0XM = S_W8M + W8H               # 1024
S_W8U = S_T0XM + HB                # 3072
S_T0XU = S_W8U + W8H               # 4096
S_T0R = S_T0XU + HB                # 6144
S_T = lambda t: S_T0R + TB + (t - 1) * 2 * TB  # x_t at +0, r_t at +TB (t>=1)
SBYTES = S_T0R + TB + 2 * TB * (NTILES - 1)    # 34816


def _chunk3(Wext, kc=8):
    """[K, M] -> [128, kc, M]: chunk k = rows k*128..(k+1)*128."""
    m = Wext.shape[1]
    return np.ascontiguousarray(Wext.reshape(kc, 128, m).transpose(1, 0, 2))


def _col(vec):
    out = np.zeros((128, 1), np.float32)
    out[: len(vec), 0] = vec
    return out


def _pack_weights(Wm, bm, Wu, bu, W1, b1, W2, b2, W3, b3):
    f64 = np.float64
    fp8 = lambda a: np.asarray(a, ml_dtypes.float8_e4m3).astype(f64)
    Wm, bm, Wu, bu = Wm.astype(f64), bm.astype(f64), Wu.astype(f64), bu.astype(f64)
    W1, b1, W2, b2 = W1.astype(f64), b1.astype(f64), W2.astype(f64), b2.astype(f64)
    b3v = float(np.asarray(b3, f64).reshape(-1)[0])

    Am = Wm[:, :256] @ W1[:256, :]
    Au = Wu[:, :256] @ W1[256:, :]
    b1p = b1 + bm[:256] @ W1[:256, :] + bu[:256] @ W1[256:, :]

    FWm = Wm[:, :256].reshape(F, 16, 16).sum(axis=1)
    FWu = Wu[:, :256].reshape(F, 16, 16).sum(axis=1)
    fbm = bm[:256].reshape(16, 16).sum(axis=0)
    fbu = bu[:256].reshape(16, 16).sum(axis=0)
    awm, awu = Wm[:, 256], Wu[:, 256]
    A = bm[256] + bu[256] + b3v
    Xm = np.concatenate([FWm, FWm, awm[:, None] / 2, awm[:, None] / 2], axis=1)
    Xu = np.concatenate([FWu, -FWu, awu[:, None] / 2, awu[:, None] / 2], axis=1)
    xbias = np.concatenate([fbm + fbu, fbm - fbu, [(A + 1) / 2], [(A - 1) / 2]])
    wq = np.concatenate([np.full(16, 0.25), np.full(16, -0.25), [1.0, -1.0]])

    # fp8 z1 blob [128, 2(half), 2(g), KC, 128], scaled x16
    amc, auc = _chunk3(Am, KC), _chunk3(Au, KC)  # [128, 4, 256]
    w8 = np.stack(
        [
            np.stack([amc[:, :, :128], amc[:, :, 128:]], axis=1),
            np.stack([auc[:, :, :128], auc[:, :, 128:]], axis=1),
        ],
        axis=1,
    )  # [128, half, g, KC, 128]
    w8 = (w8 * Z1_SCALE).astype(ml_dtypes.float8_e4m3)

    # fp8 extras blob [128, NCH, 4, 2, XW]: chains c0=F8, c1=F8/RS, c2=Fr8;
    # instruction j holds the DoubleRow chunk-pair (2j, 2j+1), cols 34:XW pad
    XWmat = np.concatenate([Xm, Xu], axis=0)          # [1024, 34]
    F8 = fp8(XWmat * XG)
    Fr = XWmat * XG - F8
    wx8 = np.zeros((128, NCH, 4, 2, XW), f64)
    for c, mat in enumerate([F8, F8 / RS, Fr]):
        wx8[:, c, :, :, :NX] = _chunk3(mat).reshape(128, 4, 2, NX)
    wx8 = wx8.astype(ml_dtypes.float8_e4m3)

    w28 = (_chunk3(W2, 2) * Z1_SCALE).astype(ml_dtypes.float8_e4m3)
    wp = np.zeros((128, WCOLS), f64)
    wp[:, W3_COL] = np.asarray(W3, f64).reshape(128) / Z1_SCALE
    wp[:NX, WQ_COL] = wq
    bp = np.concatenate(
        [_col(b1p[:128]), _col(b1p[128:]), _col(xbias), _col(b2 * Z1_SCALE)], axis=1
    )

    # z1 weights ride the front of the main consumption-ordered stream;
    # the rest is byte-packed into a small secondary blob
    wall = np.zeros((128, WBYTES), np.uint8)
    wall[:, O_WX:O_W28] = wx8.reshape(128, WX_BYTES).view(np.uint8)
    wall[:, O_W28:O_WP] = w28.reshape(128, 256).view(np.uint8)
    wall[:, O_WP:O_BP] = (
        np.ascontiguousarray(wp.astype(ml_dtypes.bfloat16)).view(np.uint8)
    )
    wall[:, O_BP:WBYTES] = (
        np.ascontiguousarray(bp.astype(np.float32)).view(np.uint8)
    )
    return (
        np.ascontiguousarray(w8.reshape(128, Z1_COLS)),
        np.ascontiguousarray(wall.view(ml_dtypes.float8_e4m3)),
    )


def _build_bass():
    nc = bacc.Bacc()
    xrd = nc.dram_tensor("xr", [128, SBYTES], E4M3, kind="ExternalInput")
    walld = nc.dram_tensor("wall", [128, WBYTES], E4M3, kind="ExternalInput")
    out = nc.dram_tensor("out", [1, R], F32, kind="ExternalOutput")

    relu = mybir.ActivationFunctionType.Relu
    square = mybir.ActivationFunctionType.Square
    DR = mybir.MatmulPerfMode.DoubleRow

    with tile.TileContext(nc) as tc:
        with (
            tc.tile_pool(name="sb", bufs=1) as sbp,
            tc.tile_pool(name="ps", bufs=1, space="PSUM") as psp,
        ):
            # explicit PSUM tiles, reused across batch tiles (ping-pong on
            # t%2): tile-release bookkeeping is what the framework's
            # end-of-kernel semaphore sweep scales with, so allocate ONCE.
            # 8 banks: 4 z1 (2 groups x 2 phases) + 2 extras + mlp2 + final
            pz = [psp.tile([128, NT], F32, name=f"pz{i}") for i in range(4)]
            px = [psp.tile([XW, NT], F32, name=f"px{i}") for i in range(2)]
            pm = psp.tile([128, NT], F32, name="pm")
            pf = psp.tile([1, NT], F32, name="pf")

            # PE pre-warm (see N_WARM note)
            wgar = sbp.tile([128, NT], BF16)
            nc.gpsimd.memset(wgar, 0.0)
            for i in range(N_WARM):
                nc.tensor.matmul(
                    pz[i % 4][:, :256], wgar[:, :128], wgar[:, :256],
                    start=True, stop=True,
                )

            # secondary weight blob (extras/mlp weights + biases, 179KB) in
            # one wide-line DMA on the scalar ring; done well before the
            # main stream needs attention
            wall_sb = sbp.tile([128, WBYTES], E4M3)
            nc.scalar.dma_start(out=wall_sb, in_=walld[:, :])
            wx8 = wall_sb[:, O_WX : O_WX + WX_BYTES].rearrange(
                "p (c j k m) -> p c j k m", c=NCH, j=4, k=2, m=XW
            )
            w28 = wall_sb[:, O_W28 : O_W28 + 256].rearrange(
                "p (k m) -> p k m", k=2, m=128
            )
            wrm = wall_sb[:, O_WP : O_WP + 2 * WCOLS].bitcast(BF16)
            b = wall_sb[:, O_BP : O_BP + 4 * BCOLS].bitcast(F32)
            out_sb = sbp.tile([1, NTILES * NT], F32)

            # THE main stream: one queue (sync ring), strict consumption
            # order, so all DMA bandwidth always serves the next-needed
            # transfer.  Eleven dma_starts (0.13-0.5MB each) keep the
            # 3-deep descriptor ring from ever running dry while giving
            # fine-grained completion at the head (z1(0) m-half starts
            # after just w8m+x0m).
            w8m = sbp.tile([128, 2, KC, 128], E4M3, name="w8m")
            nc.sync.dma_start(out=w8m, in_=xrd[:, S_W8M : S_W8M + W8H])
            x0m = sbp.tile([128, KC, NT], E4M3, name="x0m")
            nc.sync.dma_start(out=x0m, in_=xrd[:, S_T0XM : S_T0XM + HB])
            w8u = sbp.tile([128, 2, KC, 128], E4M3, name="w8u")
            nc.sync.dma_start(out=w8u, in_=xrd[:, S_W8U : S_W8U + W8H])
            x0u = sbp.tile([128, KC, NT], E4M3, name="x0u")
            nc.sync.dma_start(out=x0u, in_=xrd[:, S_T0XU : S_T0XU + HB])
            r0 = sbp.tile([128, 2, KC, NT], E4M3, name="r8_0")
            nc.sync.dma_start(out=r0, in_=xrd[:, S_T0R : S_T0R + TB])
            xts = [sbp.tile([128, 2, KC, NT], E4M3, name=f"x8_{t}")
                   for t in range(1, NTILES)]
            rts = [sbp.tile([128, 2, KC, NT], E4M3, name=f"r8_{t}")
                   for t in range(1, NTILES)]
            for t in range(1, NTILES):
                nc.sync.dma_start(
                    out=xts[t - 1], in_=xrd[:, S_T(t) : S_T(t) + TB]
                )
                nc.sync.dma_start(
                    out=rts[t - 1], in_=xrd[:, S_T(t) + TB : S_T(t) + 2 * TB]
                )
            x8ts = [(x0m, x0u)] + [(x[:, 0], x[:, 1]) for x in xts]
            r8ts = [(r0[:, 0], r0[:, 1])] + [(r[:, 0], r[:, 1]) for r in rts]

            # double-buffered activation outputs, reused on t%2
            h1s = [sbp.tile([128, 2, NT], E4M3, name=f"h1_{i}") for i in range(2)]
            sqs = [sbp.tile([XW, NT], BF16, name=f"sq_{i}") for i in range(2)]
            h2s = [sbp.tile([128, NT], BF16, name=f"h2_{i}") for i in range(2)]

            def emit_z1(t):
                # fp8 DoubleRow: each matmul contracts K=256 (two chunks as
                # a [128, 2, *] AP). xm feeds both output groups first so
                # the xu input can land meanwhile.
                xm8, xu8 = x8ts[t]
                pss = [pz[2 * (t % 2) + g] for g in range(2)]
                for half, xf in ((0, xm8), (1, xu8)):
                    w8t = (w8m, w8u)[half]
                    for g in range(2):
                        for p in range(KC // 2):
                            nc.tensor.matmul(
                                pss[g],
                                w8t[:, g, 2 * p : 2 * p + 2, :],
                                xf[:, 2 * p : 2 * p + 2, :],
                                start=(half == 0 and p == 0),
                                stop=(half == 1 and p == KC // 2 - 1),
                                perf_mode=DR,
                            )
                        if half == 1:
                            nc.scalar.activation(
                                out=h1s[t % 2][:, g, :], in_=pss[g], func=relu,
                                bias=b[:, g : g + 1], scale=1.0 / Z1_SCALE,
                            )

            def emit_extras(t):
                # three DoubleRow chains accumulate G * (XW^T x) in fp32:
                # F8^T x8 + (F8/RS)^T r8 + Fr8^T x8; the Square's
                # scale=1/G recovers the true pre-activation.
                ps = px[t % 2]
                first, last = (0, 0, 0), (NCH - 1, 1, KC // 2 - 1)
                for c in range(NCH):
                    src = r8ts[t] if c == 1 else x8ts[t]
                    for tw in range(2):
                        xf = src[tw]
                        for p in range(KC // 2):
                            nc.tensor.matmul(
                                ps,
                                wx8[:, c, tw * (KC // 2) + p, :, :],
                                xf[:, 2 * p : 2 * p + 2, :],
                                start=((c, tw, p) == first),
                                stop=((c, tw, p) == last),
                                perf_mode=DR,
                            )
                nc.scalar.activation(
                    out=sqs[t % 2], in_=ps, func=square,
                    bias=b[:XW, BX : BX + 1], scale=1.0 / XG,
                )

            def emit_mlp2(t):
                # one fp8 DoubleRow matmul (K=256); W2 scaled x16, undone by
                # the relu's scale
                nc.tensor.matmul(
                    pm, w28[:, 0:2, :], h1s[t % 2][:, 0:2, :],
                    start=True, stop=True, perf_mode=DR,
                )
                nc.vector.tensor_scalar(
                    out=h2s[t % 2], in0=pm, scalar1=b[:, B2C : B2C + 1],
                    scalar2=0.0,
                    op0=mybir.AluOpType.add, op1=mybir.AluOpType.max,
                )

            def emit_final(t):
                # sq matmul first: its operand is ready well before h2.
                # M=1 lhsT columns; the wq matmul contracts only the 34
                # real extras rows so the pad rows are never read.
                nc.tensor.matmul(
                    pf, wrm[:NX, WQ_COL : WQ_COL + 1], sqs[t % 2][:NX, :],
                    start=True, stop=False,
                )
                nc.tensor.matmul(
                    pf, wrm[:, W3_COL : W3_COL + 1], h2s[t % 2],
                    start=False, stop=True,
                )
                n0 = t * NT
                ob = out_sb[0:1, n0 : n0 + NT]
                nc.vector.tensor_scalar(
                    out=ob, in0=pf[0:1], scalar1=0.0, scalar2=0.0,
                    op0=mybir.AluOpType.add, op1=mybir.AluOpType.bypass,
                )

            for t in range(NTILES):
                emit_z1(t)
                if t > 0:
                    emit_mlp2(t - 1)
                emit_extras(t)
                if t > 0:
                    emit_final(t - 1)
            emit_mlp2(NTILES - 1)
            emit_final(NTILES - 1)
            # single output DMA: everything is staged in out_sb (scalar
            # ring, so the sync engine's stream ends early)
            nc.scalar.dma_start(out=out[:, :], in_=out_sb)
    nc.finalize()
    return nc


def _pack_x(xmT_core, xuT_core, w8):
    """2x [512, 2048] fp32 + w8 -> [128, SBYTES] consumption-ordered fp8
    stream [w8 | x0 | r0 | x1 r1 | x2 r2 | x3 r3]."""
    ym = xmT_core.reshape(KC, 128, NTILES, NT).transpose(1, 2, 0, 3)
    yu = xuT_core.reshape(KC, 128, NTILES, NT).transpose(1, 2, 0, 3)
    y = np.stack([ym, yu], axis=2)  # [128, NTILES, 2, KC, NT]
    x8 = y.astype(ml_dtypes.float8_e4m3)
    r8 = ((y - x8.astype(np.float32)) * RS).astype(ml_dtypes.float8_e4m3)
    xr = np.empty((128, SBYTES), ml_dtypes.float8_e4m3)
    xr[:, S_W8M : S_W8M + W8H] = w8[:, :W8H]
    xr[:, S_T0XM : S_T0XM + HB] = x8[:, 0, 0].reshape(128, HB)
    xr[:, S_W8U : S_W8U + W8H] = w8[:, W8H:]
    xr[:, S_T0XU : S_T0XU + HB] = x8[:, 0, 1].reshape(128, HB)
    xr[:, S_T0R : S_T0R + TB] = r8[:, 0].reshape(128, TB)
    for t in range(1, NTILES):
        xr[:, S_T(t) : S_T(t) + TB] = x8[:, t].reshape(128, TB)
        xr[:, S_T(t) + TB : S_T(t) + 2 * TB] = r8[:, t].reshape(128, TB)
    return xr


_NC_CACHE = []


def kernel(movie_vectors, user_vectors, Wm, bm, Wu, bu, W1, b1, W2, b2, W3, b3):
    movie_vectors = np.asarray(movie_vectors, np.float32)
    user_vectors = np.asarray(user_vectors, np.float32)
    w8, wall = _pack_weights(
        np.asarray(Wm, np.float32), np.asarray(bm, np.float32),
        np.asarray(Wu, np.float32), np.asarray(bu, np.float32),
        np.asarray(W1, np.float32), np.asarray(b1, np.float32),
        np.asarray(W2, np.float32), np.asarray(b2, np.float32),
        np.asarray(W3, np.float32), np.asarray(b3, np.float32),
    )
    xmT = movie_vectors.T  # [512, 16384]
    xuT = user_vectors.T

    if not _NC_CACHE:
        _NC_CACHE.append(_build_bass())
    nc = _NC_CACHE[0]

    in_maps = []
    for c in range(N_CORES):
        sl = slice(c * R, (c + 1) * R)
        xr = _pack_x(xmT[:, sl], xuT[:, sl], w8)
        in_maps.append({"xr": xr, "wall": wall})
    res = run_bass_kernel_spmd(nc, in_maps, core_ids=list(range(N_CORES)))
    kernel.last_result = res
    return np.concatenate([r["out"].reshape(R, 1) for r in res.results], axis=0)
